# revision 1
# baseline (speedup 1.0000x reference)
"""Trainium2 Bass kernel for a dense transformer block (LN -> 16-head causal
attention -> residual -> LN -> FFN -> residual) on x:(2, 2048, 1024) fp32.

Sharding: 8 cores, zero collectives. Core c handles batch b=c//4, query chunk
a=c%4 (512 contiguous tokens). Every core recomputes full-sequence K/V for its
batch from a replicated (transposed) copy of x[b]; attention for the 512
queries runs against all 2048 keys with an additive causal mask supplied as
per-core input data, so the compiled program is identical across cores (SPMD).

Everything on-chip runs feature-on-partition ("T-layout"): LayerNorm statistics
are partition reductions done with ones-vector matmuls, the softmax denominator
comes from a ones column appended to V, and per-token stats are broadcast back
across partitions with gpsimd.partition_broadcast. Matmuls run in bf16 with
fp32 PSUM accumulation; both residual adds are carried in fp32.
"""

import numpy as np
import ml_dtypes

import concourse.bass as bass
import concourse.tile as tile
from concourse import bacc, mybir
from concourse import bass_utils
from concourse.bass import ts

P = 128
B, T, C = 2, 2048, 1024
H, D = 16, 64
FF = 4 * C
CC = C // P          # 8 feature chunks
TQ = 512             # queries per core
NSCH = T // P        # 16 key chunks
EPS = 1e-5
NEG = -30000.0
bf16 = ml_dtypes.bfloat16

f32 = mybir.dt.float32
bf = mybir.dt.bfloat16
AF = mybir.ActivationFunctionType
ALU = mybir.AluOpType


def _ln_T(nc, big1, chunked, spsum, x_tile, Tn, out_tile, g_sb, be_sb, eps11,
          ones1, x_is_f32):
    """LayerNorm over the feature dim with activations feature-on-partition.
    x_tile/out_tile: (P, CC, Tn). Stats via ones-matmul partition reduction,
    processed 512 tokens at a time. big1: bufs=1 pool; chunked: bufs>=2."""
    for tch in range(Tn // 512):
        xs = x_tile[:, :, ts(tch, 512)]
        ps = spsum.tile([1, 512], f32, tag="stat")
        if x_is_f32:
            for cc in range(CC):
                xbf = chunked.tile([P, 512], bf, tag="ln_xbf")
                nc.vector.tensor_copy(xbf, xs[:, cc, :])
                nc.tensor.matmul(ps, lhsT=ones1, rhs=xbf,
                                 start=(cc == 0), stop=(cc == CC - 1))
        else:
            for cc in range(CC):
                nc.tensor.matmul(ps, lhsT=ones1, rhs=xs[:, cc, :],
                                 start=(cc == 0), stop=(cc == CC - 1))
        pq = spsum.tile([1, 512], f32, tag="stat")
        for cc in range(CC):
            sq = chunked.tile([P, 512], bf, tag="ln_sq")
            nc.vector.tensor_mul(sq, xs[:, cc, :], xs[:, cc, :])
            nc.tensor.matmul(pq, lhsT=ones1, rhs=sq,
                             start=(cc == 0), stop=(cc == CC - 1))
        m = big1.tile([1, 512], f32, tag="ln_m")
        nc.vector.tensor_scalar_mul(m, ps, 1.0 / C)
        q = big1.tile([1, 512], f32, tag="ln_q")
        nc.vector.tensor_scalar_mul(q, pq, 1.0 / C)
        msq = big1.tile([1, 512], f32, tag="ln_msq")
        nc.vector.tensor_mul(msq, m, m)
        nc.vector.tensor_tensor(q, q, msq, ALU.subtract)  # q := var
        sd = big1.tile([1, 512], f32, tag="ln_sd")
        nc.scalar.activation(sd, q, AF.Sqrt, bias=eps11)
        a_t = big1.tile([1, 512], f32, tag="ln_at")
        nc.vector.reciprocal(a_t, sd)
        b_t = big1.tile([1, 512], f32, tag="ln_bt")
        nc.vector.tensor_mul(b_t, m, a_t)

        a_bc = chunked.tile([P, 512], f32, tag="ln_abc")
        nc.gpsimd.partition_broadcast(a_bc, a_t)
        b_bc = chunked.tile([P, 512], f32, tag="ln_bbc")
        nc.gpsimd.partition_broadcast(b_bc, b_t)
        for cc in range(CC):
            t1 = chunked.tile([P, 512], bf, tag="ln_t1")
            nc.vector.tensor_mul(t1, xs[:, cc, :], a_bc)
            nc.vector.tensor_tensor(t1, t1, b_bc, ALU.subtract)
            nc.vector.tensor_scalar(out_tile[:, cc, ts(tch, 512)], t1,
                                    scalar1=g_sb[:, cc:cc + 1],
                                    scalar2=be_sb[:, cc:cc + 1],
                                    op0=ALU.mult, op1=ALU.add)


FKV = 8 * 512 + 4 * H * 65          # AllGather payload per core (bf16 elems)


def _body(nc, tc, aps, use_ag, bounces):
    (xkvT, xqT, maskT, wq, wk, wv, wo, w1, w2,
     bo_t, b1_t, b2_t, g1_t, be1_t, g2_t, be2_t, outT) = aps

    import contextlib
    ctx = contextlib.ExitStack()
    with ctx:
        # pools that live for the whole kernel (small stuff + psum)
        consts = ctx.enter_context(tc.tile_pool(name="consts", bufs=1))
        small = ctx.enter_context(tc.tile_pool(name="small", bufs=2))
        ppool = ctx.enter_context(tc.tile_pool(name="ppool", bufs=5, space="PSUM"))
        opsum = ctx.enter_context(tc.tile_pool(name="opsum", bufs=1, space="PSUM"))
        spsum = ctx.enter_context(tc.tile_pool(name="spsum", bufs=2, space="PSUM"))

        ones1 = consts.tile([P, 1], bf)
        nc.vector.memset(ones1, 1.0)
        eps11 = consts.tile([1, 1], f32)
        nc.vector.memset(eps11, EPS)

        def load(pool, ap_dram, shape, dtype=f32, tag=None):
            t = pool.tile(list(shape), dtype, tag=tag or ap_dram.name)
            nc.sync.dma_start(t, ap_dram)
            return t

        bo_s = load(consts, bo_t, (P, CC))
        b1_s = load(consts, b1_t, (P, 32))
        b2_s = load(consts, b2_t, (P, CC))
        g1_s = load(consts, g1_t, (P, CC))
        be1_s = load(consts, be1_t, (P, CC))
        g2_s = load(consts, g2_t, (P, CC))
        be2_s = load(consts, be2_t, (P, CC))

        # ---- OT survives until the output projection (opened first: LIFO)
        opool = ctx.enter_context(tc.tile_pool(name="opool", bufs=1))

        # ---- KT/Vr/QT live from the projections to the end of attention
        kvq_ctx = contextlib.ExitStack()
        kvq = kvq_ctx.enter_context(tc.tile_pool(name="kvq", bufs=1))
        KT = kvq.tile([P, 8, T], bf)
        Vr = kvq.tile([P, NSCH, H, 65], bf)
        QT = kvq.tile([P, 8, TQ], bf)

        # ---- hkv/hq live until the end of the QKV projections
        with tc.tile_pool(name="hpool", bufs=1) as hpool:
            hq = hpool.tile([P, CC, TQ], bf)

            if not use_ag:
                hkv = hpool.tile([P, CC, T], bf)
                # LN1 over the full batch sequence (for K/V)
                with tc.tile_pool(name="p_ln1", bufs=1) as p1, \
                     tc.tile_pool(name="p_ln1b", bufs=1) as p1b, \
                     tc.tile_pool(name="p_ln1t", bufs=3) as p1t:
                    xkv_sb = load(p1, xkvT, (P, CC, T), bf)
                    _ln_T(nc, p1b, p1t, spsum, xkv_sb, T, hkv,
                          g1_s, be1_s, eps11, ones1, x_is_f32=False)

            # LN1 over the query slice
            with tc.tile_pool(name="p_ln1q", bufs=1) as p2, \
                 tc.tile_pool(name="p_ln1qb", bufs=1) as p2b, \
                 tc.tile_pool(name="p_ln1qt", bufs=2) as p2t:
                xq_sb = load(p2, xqT, (P, CC, TQ), f32, tag="xq_ln")
                _ln_T(nc, p2b, p2t, spsum, xq_sb, TQ, hq,
                      g1_s, be1_s, eps11, ones1, x_is_f32=True)

            with tc.tile_pool(name="p_w", bufs=1) as pw:
                wq_s = load(pw, wq, (P, CC, 8, P), bf)
                wk_s = load(pw, wk, (P, CC, 8, P), bf)
                wv_s = load(pw, wv, (P, CC, C), bf)

                if use_ag:
                    kv_in, kv_out = bounces
                    # own-chunk K^T (s = this core's 512 tokens)
                    KTo = pw.tile([P, CC, TQ], bf, tag="KTo")
                    for pair in range(8):
                        psum = ppool.tile([P, 512], f32, tag="mm")
                        for cc in range(CC):
                            nc.tensor.matmul(psum, lhsT=wk_s[:, cc, pair, :],
                                             rhs=hq[:, cc, :],
                                             start=(cc == 0), stop=(cc == CC - 1))
                        nc.vector.tensor_copy(KTo[:, pair, :], psum)
                    # own-chunk V rows (4 s-tiles) with ones column
                    Vro = pw.tile([P, 4, H, 65], bf, tag="Vro")
                    nc.vector.memset(Vro[:, :, :, 64:65], 1.0)
                    for st in range(4):
                        for half in range(2):
                            psum = ppool.tile([P, 512], f32, tag="mm")
                            for cc in range(CC):
                                nc.tensor.matmul(psum, lhsT=hq[:, cc, ts(st, P)],
                                                 rhs=wv_s[:, cc, ts(half, 512)],
                                                 start=(cc == 0), stop=(cc == CC - 1))
                            nc.vector.tensor_copy(
                                Vro[:, st, half * 8:(half + 1) * 8, 0:64],
                                psum.rearrange("p (h d) -> p h d", d=64))
                    # bounce out, AllGather within the 4-core batch group,
                    # then scatter the gathered chunks into KT / Vr
                    nc.sync.dma_start(
                        kv_in.ap()[:, 0:4096].rearrange("p (a b) -> p a b", a=CC),
                        KTo)
                    nc.sync.dma_start(
                        kv_in.ap()[:, 4096:FKV].rearrange(
                            "p (a h e) -> p a h e", a=4, h=H), Vro)
                    nc.gpsimd.collective_compute(
                        "AllGather",
                        mybir.AluOpType.bypass,
                        replica_groups=[[0, 1, 2, 3], [4, 5, 6, 7]],
                        ins=[kv_in.ap().opt()],
                        outs=[kv_out.ap().opt()],
                    )
                    for r in range(4):
                        blk = kv_out.ap()[r * P:(r + 1) * P, :]
                        nc.sync.dma_start(
                            KT[:, :, ts(r, 512)],
                            blk[:, 0:4096].rearrange("p (a b) -> p a b", a=CC))
                        nc.sync.dma_start(
                            Vr[:, 4 * r:4 * (r + 1), :, :],
                            blk[:, 4096:FKV].rearrange(
                                "p (a h e) -> p a h e", a=4, h=H))
                else:
                    for pair in range(8):
                        for sch in range(4):
                            psum = ppool.tile([P, 512], f32, tag="mm")
                            for cc in range(CC):
                                nc.tensor.matmul(psum, lhsT=wk_s[:, cc, pair, :],
                                                 rhs=hkv[:, cc, ts(sch, 512)],
                                                 start=(cc == 0), stop=(cc == CC - 1))
                            nc.vector.tensor_copy(KT[:, pair, ts(sch, 512)], psum)

                    nc.vector.memset(Vr[:, :, :, 64:65], 1.0)
                    for st in range(NSCH):
                        for half in range(2):
                            psum = ppool.tile([P, 512], f32, tag="mm")
                            for cc in range(CC):
                                nc.tensor.matmul(psum, lhsT=hkv[:, cc, ts(st, P)],
                                                 rhs=wv_s[:, cc, ts(half, 512)],
                                                 start=(cc == 0), stop=(cc == CC - 1))
                            nc.vector.tensor_copy(
                                Vr[:, st, half * 8:(half + 1) * 8, 0:64],
                                psum.rearrange("p (h d) -> p h d", d=64))

                for pair in range(8):
                    psum = ppool.tile([P, 512], f32, tag="mm")
                    for cc in range(CC):
                        nc.tensor.matmul(psum, lhsT=wq_s[:, cc, pair, :],
                                         rhs=hq[:, cc, :],
                                         start=(cc == 0), stop=(cc == CC - 1))
                    nc.vector.tensor_scalar_mul(QT[:, pair, :], psum,
                                                float(C) ** -0.5)

        # ---- attention
        OT = opool.tile([P, 8, TQ], bf)
        with tc.tile_pool(name="p_att", bufs=1) as pa, \
             tc.tile_pool(name="p_attt", bufs=4) as pat:
            mask_sb = load(pa, maskT, (P, NSCH, TQ), bf)
            for h in range(H):
                pair, half = h // 2, h % 2
                hp = slice(64 * half, 64 * half + 64)
                ops = opsum.tile([P, 512], f32, tag="av")
                for sch in range(NSCH):
                    sps = ppool.tile([P, 512], f32, tag="mm")
                    nc.tensor.matmul(sps, lhsT=KT[hp, pair, ts(sch, P)],
                                     rhs=QT[hp, pair, :], start=True, stop=True)
                    sm = pat.tile([P, TQ], bf, tag="sm")
                    nc.vector.tensor_tensor(sm, sps, mask_sb[:, sch, :], ALU.add)
                    e = pat.tile([P, TQ], bf, tag="e")
                    nc.scalar.activation(e, sm, AF.Exp)
                    nc.tensor.matmul(ops[0:65, :], lhsT=Vr[:, sch, h, :], rhs=e,
                                     start=(sch == 0), stop=(sch == NSCH - 1))
                zr = small.tile([1, TQ], f32, tag="zr")
                nc.vector.reciprocal(zr, ops[64:65, :])
                zb = pat.tile([64, TQ], f32, tag="zb")
                nc.gpsimd.partition_broadcast(zb, zr)
                nc.vector.tensor_mul(OT[hp, pair, :], ops[0:64, :], zb)
        kvq_ctx.close()

        # ---- output projection + bias + residual; LN2; FFN
        with tc.tile_pool(name="p_ffn", bufs=1) as pf, \
             tc.tile_pool(name="p_ffnt", bufs=2) as pft, \
             tc.tile_pool(name="p_wstream", bufs=3) as pws:
            xq_sb = load(pf, xqT, (P, CC, TQ), f32, tag="xq_res")
            wo_s = load(pf, wo, (P, CC, 8, P), bf)
            y1 = pf.tile([P, CC, TQ], f32)
            for mo in range(CC):
                psum = ppool.tile([P, 512], f32, tag="mm")
                for cc in range(CC):
                    nc.tensor.matmul(psum, lhsT=wo_s[:, cc, mo, :],
                                     rhs=OT[:, cc, :],
                                     start=(cc == 0), stop=(cc == CC - 1))
                t = pft.tile([P, TQ], f32, tag="res")
                nc.vector.tensor_scalar_add(t, psum, bo_s[:, mo:mo + 1])
                nc.vector.tensor_tensor(y1[:, mo, :], t, xq_sb[:, mo, :], ALU.add)

            h2 = pf.tile([P, CC, TQ], bf)
            with tc.tile_pool(name="p_ln2b", bufs=1) as pl2b:
                _ln_T(nc, pl2b, pft, spsum, y1, TQ, h2,
                      g2_s, be2_s, eps11, ones1, x_is_f32=True)

            zT = pf.tile([P, 32, TQ], bf)
            for m in range(32):
                w1b = pws.tile([P, CC, P], bf, tag="w1")
                nc.sync.dma_start(w1b, w1[m])
                psum = ppool.tile([P, 512], f32, tag="mm")
                for cc in range(CC):
                    nc.tensor.matmul(psum, lhsT=w1b[:, cc, :], rhs=h2[:, cc, :],
                                     start=(cc == 0), stop=(cc == CC - 1))
                nc.scalar.activation(zT[:, m, :], psum, AF.Relu,
                                     bias=b1_s[:, m:m + 1])

            for mo in range(CC):
                w2b = pws.tile([P, 32, P], bf, tag="w2")
                nc.sync.dma_start(w2b, w2[mo])
                psum = ppool.tile([P, 512], f32, tag="mm")
                for ff in range(32):
                    nc.tensor.matmul(psum, lhsT=w2b[:, ff, :], rhs=zT[:, ff, :],
                                     start=(ff == 0), stop=(ff == 31))
                t = pft.tile([P, TQ], f32, tag="res")
                nc.vector.tensor_scalar_add(t, psum, b2_s[:, mo:mo + 1])
                ot = pft.tile([P, TQ], f32, tag="ot")
                nc.vector.tensor_tensor(ot, t, y1[:, mo, :], ALU.add)
                nc.sync.dma_start(outT[:, mo, :], ot)


_NC_CACHE = {}
USE_AG = False


def build_nc(reps=1, use_ag=None):
    global _NC_CACHE
    if use_ag is None:
        use_ag = USE_AG
    key = (reps, use_ag)
    if key in _NC_CACHE:
        return _NC_CACHE[key]
    nc = bacc.Bacc("TRN2", target_bir_lowering=False, debug=False,
                   enable_asserts=False, num_devices=8)

    def dram(name, shape, dtype, kind="ExternalInput"):
        return nc.dram_tensor(name, shape, dtype, kind=kind).ap()

    aps = (
        dram("xkvT", (P, CC, T), bf) if not use_ag else None,
        dram("xqT", (P, CC, TQ), f32),
        dram("maskT", (P, NSCH, TQ), bf),
        dram("wq", (P, CC, 8, P), bf),
        dram("wk", (P, CC, 8, P), bf),
        dram("wv", (P, CC, C), bf),
        dram("wo", (P, CC, 8, P), bf),
        dram("w1", (32, P, CC, P), bf),
        dram("w2", (CC, P, 32, P), bf),
        dram("bo_t", (P, CC), f32),
        dram("b1_t", (P, 32), f32),
        dram("b2_t", (P, CC), f32),
        dram("g1_t", (P, CC), f32),
        dram("be1_t", (P, CC), f32),
        dram("g2_t", (P, CC), f32),
        dram("be2_t", (P, CC), f32),
        dram("outT", (P, CC, TQ), f32, kind="ExternalOutput"),
    )
    bounces = None
    if use_ag:
        bounces = []
        for i in range(reps):
            kv_in = nc.dram_tensor(f"kv_in{i}", (P, FKV), bf)
            kv_out = nc.dram_tensor(f"kv_out{i}", (4 * P, FKV), bf)
            bounces.append((kv_in, kv_out))
    with tile.TileContext(nc) as tc:
        for i in range(reps):
            _body(nc, tc, aps, use_ag, bounces[i] if use_ag else None)
    nc.compile()
    _NC_CACHE[key] = nc
    return nc


def _tile_lhst(w):  # (C, C) -> (P, cc, pair/mo, 128)
    return np.ascontiguousarray(
        w.reshape(CC, P, 8, P).transpose(1, 0, 2, 3)).astype(bf16)


def make_in_maps(inputs, use_ag=None):
    """Build the 8 per-core input dicts from the full problem inputs."""
    if use_ag is None:
        use_ag = USE_AG
    x = np.asarray(inputs["x"], np.float32)
    Wq = np.asarray(inputs["Wq"], np.float32)
    Wk = np.asarray(inputs["Wk"], np.float32)
    Wv = np.asarray(inputs["Wv"], np.float32)
    Wo = np.asarray(inputs["Wo"], np.float32)
    W1 = np.asarray(inputs["W1"], np.float32)
    W2 = np.asarray(inputs["W2"], np.float32)

    wq_flat = np.ascontiguousarray(Wq.transpose(1, 0, 2)).reshape(C, C)
    wk_flat = np.ascontiguousarray(Wk.transpose(1, 0, 2)).reshape(C, C)
    wv_flat = np.ascontiguousarray(Wv.transpose(1, 0, 2)).reshape(C, C)

    shared = {
        "wq": _tile_lhst(wq_flat),
        "wk": _tile_lhst(wk_flat),
        "wv": np.ascontiguousarray(
            wv_flat.reshape(CC, P, C).transpose(1, 0, 2)).astype(bf16),
        "wo": _tile_lhst(Wo),
        "w1": np.ascontiguousarray(
            W1.reshape(CC, P, 32, P).transpose(2, 1, 0, 3)).astype(bf16),
        "w2": np.ascontiguousarray(
            W2.reshape(32, P, CC, P).transpose(2, 1, 0, 3)).astype(bf16),
        "bo_t": np.ascontiguousarray(
            np.asarray(inputs["bo"], np.float32).reshape(CC, P).T),
        "b1_t": np.ascontiguousarray(
            np.asarray(inputs["b1"], np.float32).reshape(32, P).T),
        "b2_t": np.ascontiguousarray(
            np.asarray(inputs["b2"], np.float32).reshape(CC, P).T),
        "g1_t": np.ascontiguousarray(
            np.asarray(inputs["g1"], np.float32).reshape(CC, P).T),
        "be1_t": np.ascontiguousarray(
            np.asarray(inputs["be1"], np.float32).reshape(CC, P).T),
        "g2_t": np.ascontiguousarray(
            np.asarray(inputs["g2"], np.float32).reshape(CC, P).T),
        "be2_t": np.ascontiguousarray(
            np.asarray(inputs["be2"], np.float32).reshape(CC, P).T),
    }

    s_idx = np.arange(T)
    in_maps = []
    for c in range(8):
        b, a = c // 4, c % 4
        q0 = TQ * a
        xbT = np.ascontiguousarray(x[b].T)                       # (C, T)
        xkvT = xbT.reshape(CC, P, T).transpose(1, 0, 2).astype(bf16)
        xqT = np.ascontiguousarray(
            xbT[:, q0:q0 + TQ].reshape(CC, P, TQ).transpose(1, 0, 2))
        mask = np.where(s_idx[:, None] <= (q0 + np.arange(TQ))[None, :],
                        np.float32(0.0), np.float32(NEG))
        maskT = mask.reshape(NSCH, P, TQ).transpose(1, 0, 2).astype(bf16)
        m = {
            "xqT": xqT.astype(np.float32),
            "maskT": np.ascontiguousarray(maskT),
            **shared,
        }
        if not use_ag:
            m["xkvT"] = np.ascontiguousarray(xkvT)
        in_maps.append(m)
    return in_maps


def assemble_output(core_outs):
    """core_outs: list of 8 dicts with 'outT' (P, CC, TQ) fp32."""
    out = np.zeros((B, T, C), np.float32)
    for c in range(8):
        b, a = c // 4, c % 4
        y2 = core_outs[c]["outT"].transpose(1, 0, 2).reshape(C, TQ)  # (C, TQ)
        out[b, TQ * a:TQ * (a + 1), :] = y2.T
    return out


def kernel(**inputs) -> np.ndarray:
    nc = build_nc()
    in_maps = make_in_maps(inputs)
    res = bass_utils.run_bass_kernel_spmd(nc, in_maps, core_ids=list(range(8)))
    return assemble_output(res.results)


if __name__ == "__main__":
    import reference
    inputs = {k: np.asarray(v) for k, v in reference.setup_inputs().items()}
    expected = np.asarray(reference.reference(**inputs))
    actual = kernel(**inputs)
    err = np.abs(actual - expected)
    print("absmax err:", err.max(), "scale:", np.abs(expected).max())
    print("rel fro:", np.linalg.norm(actual - expected) / np.linalg.norm(expected))



# revision 10
# speedup vs baseline: 1.8323x; 1.8323x over previous
"""Trainium2 Bass kernel for a dense transformer block (LN -> 16-head causal
attention -> residual -> LN -> FFN -> residual) on x:(2, 2048, 1024) fp32.

Sharding: 8 cores, zero collectives. Core c handles batch b=c//4 and 512
queries (chunk a=c%4). Each core recomputes full-sequence K/V for its batch.
Tokens are fed to each core PERMUTED so its own queries sit at positions
0-511; attention over keys is permutation-invariant given the matching 0/1
mask (per-core input data), so the compiled program is identical across
cores (SPMD) and the LN1 output can be sliced at a fixed offset for Q.

On-chip layout is feature-on-partition throughout. LayerNorm statistics come
from ones-vector matmuls; softmax uses a multiplicative 0/1 causal mask
applied AFTER exp (scores are tiny: |s|<~1, so exp never overflows and
masked lanes are zeroed exactly); the denominator comes from a ones column
appended to V. All projections (QKV/O/FFN) run in fp8e4m3 with DoubleRow
perf mode (2 contraction tiles per pass); weights are pre-scaled by a
power-of-two into fp8 range and descaled at PSUM readout. LN gains (g1/g2)
and the C**-0.5 score scale are folded into the weights host-side; LN biases
(be1/be2) are folded into per-feature bias vectors (the V-path bias commutes
through softmax since attention rows sum to 1, so it folds into bo).
Score/AV matmuls run in bf16. Exps process two heads per instruction from a
two-bank PSUM pair."""

import numpy as np
import ml_dtypes

import concourse.bass as bass
import concourse.tile as tile
from concourse import bacc, mybir
from concourse import bass_utils
from concourse.bass import ts

P = 128
B, T, C = 2, 2048, 1024
H, D = 16, 64
FF = 4 * C
CC = C // P          # 8 feature chunks
C2 = CC // 2         # 4 DoubleRow pair-chunks
TQ = 512             # queries per core
NSCH = T // P        # 16 key chunks
NTCH = T // TQ       # 4 token chunks of 512
EPS = 1e-5

bf16 = ml_dtypes.bfloat16
e4m3 = ml_dtypes.float8_e4m3

f32 = mybir.dt.float32
bf = mybir.dt.bfloat16
fp8 = mybir.dt.float8e4
AF = mybir.ActivationFunctionType
ALU = mybir.AluOpType
DR = mybir.MatmulPerfMode.DoubleRow

# sc_t column indices for the per-matrix fp8 descale factors
SCQ, SCK, SCV, SCO, SC1, SC2 = 0, 1, 2, 3, 4, 5


def _body(nc, tc, aps):
    (xkvT, xqT, maskT, wq8, wk8, wv8, wo8, w18, w28,
     sc_t, qb_t, kb_t, bo_t, b1_t, b2_t, outT) = aps

    import contextlib
    ctx = contextlib.ExitStack()
    with ctx:
        consts = ctx.enter_context(tc.tile_pool(name="consts", bufs=1))
        small = ctx.enter_context(tc.tile_pool(name="small", bufs=2))

        ones1 = consts.tile([P, 1], bf)
        nc.vector.memset(ones1, 1.0)
        eps11 = consts.tile([1, 1], f32)
        nc.vector.memset(eps11, EPS)

        def load(pool, ap_dram, shape, dtype=f32, tag=None):
            t = pool.tile(list(shape), dtype, tag=tag or ap_dram.name)
            nc.sync.dma_start(t, ap_dram)
            return t

        sc_s = load(consts, sc_t, (P, 8))
        qb_s = load(consts, qb_t, (P, 8))
        kb_s = load(consts, kb_t, (P, 8))
        bo_s = load(consts, bo_t, (P, CC))
        b1_s = load(consts, b1_t, (P, 32))
        b2_s = load(consts, b2_t, (P, CC))

        # ---- long-lived activation tensors (LIFO: outermost lives longest)
        ypool = ctx.enter_context(tc.tile_pool(name="ypool", bufs=1))
        y1 = ypool.tile([P, CC, TQ], f32)

        opool = ctx.enter_context(tc.tile_pool(name="opool", bufs=1))
        OT = opool.tile([P, CC, TQ], fp8)

        kvq_ctx = contextlib.ExitStack()
        kvq = kvq_ctx.enter_context(tc.tile_pool(name="kvq", bufs=1))
        KT = kvq.tile([P, CC, T], bf)
        Vr = kvq.tile([P, NSCH, H, 65], bf)
        QT = kvq.tile([P, CC, TQ], bf)
        mask_sb = kvq.tile([P, NSCH, TQ], bf)
        nc.sync.dma_start(mask_sb, maskT)
        xq_sb = kvq.tile([P, CC, TQ], f32)
        nc.sync.dma_start(xq_sb, xqT)
        nc.vector.memset(Vr[:, :, :, 64:65], 1.0)

        # ---- LN1 over the full (permuted) sequence -> hN8 fp8
        with tc.tile_pool(name="hpool", bufs=1) as hpool:
            hN8 = hpool.tile([P, C2, 2, T], fp8)

            with tc.tile_pool(name="xpool", bufs=1) as xpool, \
                 tc.tile_pool(name="lnt", bufs=2) as lnt, \
                 tc.tile_pool(name="lnb", bufs=1) as lnb, \
                 tc.tile_pool(name="spsum", bufs=2, space="PSUM") as spsum:
                xkv_sb = load(xpool, xkvT, (P, CC, T), bf)
                for tch in range(NTCH):
                    xs = xkv_sb[:, :, ts(tch, TQ)]
                    ps = spsum.tile([1, TQ], f32, tag="ps")
                    for cc in range(CC):
                        nc.tensor.matmul(ps, lhsT=ones1, rhs=xs[:, cc, :],
                                         start=(cc == 0), stop=(cc == CC - 1))
                    pq = spsum.tile([1, TQ], f32, tag="pq")
                    for c2 in range(C2):
                        sq2 = lnt.tile([P, 2, TQ], bf, tag="sq2")
                        x2 = xs[:, 2 * c2:2 * c2 + 2, :]
                        nc.vector.tensor_mul(sq2, x2, x2)
                        for i in range(2):
                            nc.tensor.matmul(pq, lhsT=ones1, rhs=sq2[:, i, :],
                                             start=(c2 == 0 and i == 0),
                                             stop=(c2 == C2 - 1 and i == 1))
                    m = lnb.tile([1, TQ], bf, tag="ln_m")
                    nc.vector.tensor_scalar_mul(m, ps, 1.0 / C)
                    q = lnb.tile([1, TQ], f32, tag="ln_q")
                    nc.vector.tensor_scalar_mul(q, pq, 1.0 / C)
                    msq = lnb.tile([1, TQ], f32, tag="ln_msq")
                    nc.vector.tensor_mul(msq, m, m)
                    nc.vector.tensor_tensor(q, q, msq, ALU.subtract)
                    sd = lnb.tile([1, TQ], f32, tag="ln_sd")
                    nc.scalar.activation(sd, q, AF.Sqrt, bias=eps11)
                    a_t = lnb.tile([1, TQ], bf, tag="ln_at")
                    with nc.allow_low_precision(reason="1/sigma in bf16"):
                        nc.vector.reciprocal(a_t, sd)
                    m_bc = lnt.tile([P, TQ], bf, tag="ln_mbc")
                    nc.gpsimd.partition_broadcast(m_bc, m)
                    a_bc = lnt.tile([P, TQ], bf, tag="ln_abc")
                    nc.gpsimd.partition_broadcast(a_bc, a_t)
                    m_b2 = m_bc.unsqueeze(1).broadcast_to((P, 2, TQ))
                    a_b2 = a_bc.unsqueeze(1).broadcast_to((P, 2, TQ))
                    for c2 in range(C2):
                        t2 = lnt.tile([P, 2, TQ], bf, tag="ln_t2")
                        nc.vector.tensor_tensor(
                            t2, xs[:, 2 * c2:2 * c2 + 2, :], m_b2, ALU.subtract)
                        nc.vector.tensor_mul(
                            hN8[:, c2, :, ts(tch, TQ)], t2, a_b2)

            # ---- K, V, Q projections (fp8 DoubleRow), K/Q via DVE, V via Pool
            with tc.tile_pool(name="pw", bufs=1) as pw, \
                 tc.tile_pool(name="ppool", bufs=4, space="PSUM") as ppool:
                wk_s = load(pw, wk8, (P, C2, 2, CC, P), fp8)
                wv_s = load(pw, wv8, (P, C2, 2, 2, TQ), fp8)
                wq_s = load(pw, wq8, (P, C2, 2, CC, P), fp8)

                for pair in range(CC):
                    for tch in range(NTCH):
                        psum = ppool.tile([P, TQ], f32, tag="mm")
                        for c2 in range(C2):
                            nc.tensor.matmul(
                                psum, lhsT=wk_s[:, c2, :, pair, :],
                                rhs=hN8[:, c2, :, ts(tch, TQ)],
                                start=(c2 == 0), stop=(c2 == C2 - 1),
                                perf_mode=DR)
                        nc.vector.tensor_scalar(
                            KT[:, pair, ts(tch, TQ)], psum,
                            scalar1=sc_s[:, SCK:SCK + 1],
                            scalar2=kb_s[:, pair:pair + 1],
                            op0=ALU.mult, op1=ALU.add)

                for st in range(NSCH):
                    for half in range(2):
                        psum = ppool.tile([P, TQ], f32, tag="mm")
                        for c2 in range(C2):
                            nc.tensor.matmul(
                                psum, lhsT=hN8[:, c2, :, ts(st, P)],
                                rhs=wv_s[:, c2, :, half, :],
                                start=(c2 == 0), stop=(c2 == C2 - 1),
                                perf_mode=DR)
                        nc.scalar.activation(
                            Vr[:, st, half * 8:(half + 1) * 8, 0:64],
                            psum.rearrange("p (h d) -> p h d", d=D),
                            AF.Copy, scale=sc_s[:, SCV:SCV + 1])

                for pair in range(CC):
                    psum = ppool.tile([P, TQ], f32, tag="mm")
                    for c2 in range(C2):
                        nc.tensor.matmul(
                            psum, lhsT=wq_s[:, c2, :, pair, :],
                            rhs=hN8[:, c2, :, 0:TQ],
                            start=(c2 == 0), stop=(c2 == C2 - 1),
                            perf_mode=DR)
                    nc.vector.tensor_scalar(
                        QT[:, pair, :], psum,
                        scalar1=sc_s[:, SCQ:SCQ + 1],
                        scalar2=qb_s[:, pair:pair + 1],
                        op0=ALU.mult, op1=ALU.add)

        # ---- attention: per head-pair, paired-bank scores -> exp -> 0/1 mask
        with tc.tile_pool(name="wfp", bufs=1) as wfp:
            wo_s = load(wfp, wo8, (P, C2, 2, CC, P), fp8)

            with tc.tile_pool(name="pat", bufs=4) as pat, \
                 tc.tile_pool(name="spool", bufs=2, space="PSUM") as spool, \
                 tc.tile_pool(name="avp", bufs=2, space="PSUM") as avp:
                for pr in range(CC):
                    h0, h1 = 2 * pr, 2 * pr + 1
                    av0 = avp.tile([P, TQ], f32, tag="av0")
                    av1 = avp.tile([P, TQ], f32, tag="av1")
                    for sch in range(NSCH):
                        sps = spool.tile([P, 2 * TQ], f32, tag="sps")
                        nc.tensor.matmul(sps[:, 0:TQ],
                                         lhsT=KT[0:64, pr, ts(sch, P)],
                                         rhs=QT[0:64, pr, :],
                                         start=True, stop=True)
                        nc.tensor.matmul(sps[:, TQ:2 * TQ],
                                         lhsT=KT[64:128, pr, ts(sch, P)],
                                         rhs=QT[64:128, pr, :],
                                         start=True, stop=True)
                        e2 = pat.tile([P, 2, TQ], bf, tag="e2")
                        nc.scalar.activation(
                            e2.rearrange("p a b -> p (a b)"), sps, AF.Exp)
                        em = pat.tile([P, 2, TQ], bf, tag="em")
                        m_b2 = mask_sb[:, sch, :].unsqueeze(1).broadcast_to(
                            (P, 2, TQ))
                        nc.vector.tensor_mul(em, e2, m_b2)
                        nc.tensor.matmul(av0[0:65, :], lhsT=Vr[:, sch, h0, :],
                                         rhs=em[:, 0, :], start=(sch == 0),
                                         stop=(sch == NSCH - 1))
                        nc.tensor.matmul(av1[0:65, :], lhsT=Vr[:, sch, h1, :],
                                         rhs=em[:, 1, :], start=(sch == 0),
                                         stop=(sch == NSCH - 1))
                    for i, av in ((0, av0), (1, av1)):
                        zr = small.tile([1, TQ], f32, tag="zr")
                        nc.vector.reciprocal(zr, av[64:65, :])
                        zb = pat.tile([64, TQ], f32, tag="zb")
                        nc.gpsimd.partition_broadcast(zb, zr)
                        nc.vector.tensor_mul(
                            OT[64 * i:64 * i + 64, pr, :], av[0:64, :], zb)

            # ---- output projection + bias + residual -> y1 (f32)
            with tc.tile_pool(name="pft", bufs=3) as pft, \
                 tc.tile_pool(name="ppool2", bufs=4, space="PSUM") as ppool2:
                for mo in range(CC):
                    psum = ppool2.tile([P, TQ], f32, tag="mm")
                    for c2 in range(C2):
                        nc.tensor.matmul(
                            psum, lhsT=wo_s[:, c2, :, mo, :],
                            rhs=OT[:, 2 * c2:2 * c2 + 2, :],
                            start=(c2 == 0), stop=(c2 == C2 - 1),
                            perf_mode=DR)
                    tb = pft.tile([P, TQ], bf, tag="ot_b")
                    nc.vector.tensor_scalar(tb, psum,
                                            scalar1=sc_s[:, SCO:SCO + 1],
                                            scalar2=bo_s[:, mo:mo + 1],
                                            op0=ALU.mult, op1=ALU.add)
                    nc.vector.tensor_tensor(y1[:, mo, :], tb, xq_sb[:, mo, :],
                                            ALU.add)
        kvq_ctx.close()

        # ---- LN2 -> h2N8 fp8; FFN; residual
        with tc.tile_pool(name="ffp", bufs=1) as ffp, \
             tc.tile_pool(name="pft", bufs=3) as pft, \
             tc.tile_pool(name="lnb2", bufs=2) as lnb2, \
             tc.tile_pool(name="spsum2", bufs=2, space="PSUM") as spsum2, \
             tc.tile_pool(name="ppool3", bufs=4, space="PSUM") as ppool3:
            w1_s = load(ffp, w18, (P, 32, C2, 2, P), fp8)
            w2_s = load(ffp, w28, (P, CC, 16, 2, P), fp8)
            h2N8 = ffp.tile([P, C2, 2, TQ], fp8)
            y1b = ffp.tile([P, CC, TQ], bf)

            for c2 in range(C2):
                nc.vector.tensor_copy(y1b[:, 2 * c2:2 * c2 + 2, :],
                                      y1[:, 2 * c2:2 * c2 + 2, :])
            ps = spsum2.tile([1, TQ], f32, tag="ps")
            for cc in range(CC):
                nc.tensor.matmul(ps, lhsT=ones1, rhs=y1b[:, cc, :],
                                 start=(cc == 0), stop=(cc == CC - 1))
            pq = spsum2.tile([1, TQ], f32, tag="pq")
            for c2 in range(C2):
                sq2 = pft.tile([P, 2, TQ], bf, tag="sq2")
                yb2 = y1b[:, 2 * c2:2 * c2 + 2, :]
                nc.vector.tensor_mul(sq2, yb2, yb2)
                for i in range(2):
                    nc.tensor.matmul(pq, lhsT=ones1, rhs=sq2[:, i, :],
                                     start=(c2 == 0 and i == 0),
                                     stop=(c2 == C2 - 1 and i == 1))
            m = lnb2.tile([1, TQ], bf, tag="ln_m")
            nc.vector.tensor_scalar_mul(m, ps, 1.0 / C)
            q = lnb2.tile([1, TQ], f32, tag="ln_q")
            nc.vector.tensor_scalar_mul(q, pq, 1.0 / C)
            msq = lnb2.tile([1, TQ], f32, tag="ln_msq")
            nc.vector.tensor_mul(msq, m, m)
            nc.vector.tensor_tensor(q, q, msq, ALU.subtract)
            sd = lnb2.tile([1, TQ], f32, tag="ln_sd")
            nc.scalar.activation(sd, q, AF.Sqrt, bias=eps11)
            a_t = lnb2.tile([1, TQ], bf, tag="ln_at")
            with nc.allow_low_precision(reason="1/sigma in bf16"):
                nc.vector.reciprocal(a_t, sd)
            m_bc = lnb2.tile([P, TQ], bf, tag="ln_mbc")
            nc.gpsimd.partition_broadcast(m_bc, m)
            a_bc = lnb2.tile([P, TQ], bf, tag="ln_abc")
            nc.gpsimd.partition_broadcast(a_bc, a_t)
            m_b2 = m_bc.unsqueeze(1).broadcast_to((P, 2, TQ))
            a_b2 = a_bc.unsqueeze(1).broadcast_to((P, 2, TQ))
            for c2 in range(C2):
                t2 = pft.tile([P, 2, TQ], bf, tag="ln_t2")
                nc.vector.tensor_tensor(t2, y1b[:, 2 * c2:2 * c2 + 2, :],
                                        m_b2, ALU.subtract)
                nc.vector.tensor_mul(h2N8[:, c2, :, :], t2, a_b2)

            zT = ffp.tile([P, 16, 2, TQ], fp8)
            for mm in range(32):
                psum = ppool3.tile([P, TQ], f32, tag="mm")
                for c2 in range(C2):
                    nc.tensor.matmul(psum, lhsT=w1_s[:, mm, c2, :, :],
                                     rhs=h2N8[:, c2, :, :],
                                     start=(c2 == 0), stop=(c2 == C2 - 1),
                                     perf_mode=DR)
                nc.scalar.activation(zT[:, mm // 2, mm % 2, :], psum, AF.Relu,
                                     bias=b1_s[:, mm:mm + 1],
                                     scale=sc_s[:, SC1:SC1 + 1])

            for mo in range(CC):
                psum = ppool3.tile([P, TQ], f32, tag="mm")
                for f2 in range(16):
                    nc.tensor.matmul(psum, lhsT=w2_s[:, mo, f2, :, :],
                                     rhs=zT[:, f2, :, :],
                                     start=(f2 == 0), stop=(f2 == 15),
                                     perf_mode=DR)
                tb = pft.tile([P, TQ], bf, tag="ffn_b")
                nc.vector.tensor_scalar(tb, psum,
                                        scalar1=sc_s[:, SC2:SC2 + 1],
                                        scalar2=b2_s[:, mo:mo + 1],
                                        op0=ALU.mult, op1=ALU.add)
                ot = pft.tile([P, TQ], f32, tag="ot")
                nc.vector.tensor_tensor(ot, tb, y1[:, mo, :], ALU.add)
                nc.sync.dma_start(outT[:, mo, :], ot)


_NC_CACHE = {}


def build_nc(reps=1):
    global _NC_CACHE
    if reps in _NC_CACHE:
        return _NC_CACHE[reps]
    nc = bacc.Bacc("TRN2", target_bir_lowering=False, debug=False,
                   enable_asserts=False, num_devices=8)

    def dram(name, shape, dtype, kind="ExternalInput"):
        return nc.dram_tensor(name, shape, dtype, kind=kind).ap()

    aps = (
        dram("xkvT", (P, CC, T), bf),
        dram("xqT", (P, CC, TQ), f32),
        dram("maskT", (P, NSCH, TQ), bf),
        dram("wq8", (P, C2, 2, CC, P), fp8),
        dram("wk8", (P, C2, 2, CC, P), fp8),
        dram("wv8", (P, C2, 2, 2, TQ), fp8),
        dram("wo8", (P, C2, 2, CC, P), fp8),
        dram("w18", (P, 32, C2, 2, P), fp8),
        dram("w28", (P, CC, 16, 2, P), fp8),
        dram("sc_t", (P, 8), f32),
        dram("qb_t", (P, CC), f32),
        dram("kb_t", (P, CC), f32),
        dram("bo_t", (P, CC), f32),
        dram("b1_t", (P, 32), f32),
        dram("b2_t", (P, CC), f32),
        dram("outT", (P, CC, TQ), f32, kind="ExternalOutput"),
    )
    with tile.TileContext(nc) as tc:
        for _ in range(reps):
            _body(nc, tc, aps)
    nc.compile()
    _NC_CACHE[reps] = nc
    return nc


def _pow2_scale(w, target=96.0):
    mx = float(np.abs(w).max())
    if mx == 0.0:
        return 1.0
    return float(2.0 ** np.floor(np.log2(target / mx)))


def _fp8(w):
    return np.clip(w, -240.0, 240.0).astype(e4m3)


def make_in_maps(inputs):
    """Build the 8 per-core input dicts from the full problem inputs."""
    x = np.asarray(inputs["x"], np.float32)
    Wq = np.asarray(inputs["Wq"], np.float32)
    Wk = np.asarray(inputs["Wk"], np.float32)
    Wv = np.asarray(inputs["Wv"], np.float32)
    Wo = np.asarray(inputs["Wo"], np.float32)
    W1 = np.asarray(inputs["W1"], np.float32)
    W2 = np.asarray(inputs["W2"], np.float32)
    bo = np.asarray(inputs["bo"], np.float32)
    b1 = np.asarray(inputs["b1"], np.float32)
    b2 = np.asarray(inputs["b2"], np.float32)
    g1 = np.asarray(inputs["g1"], np.float32)
    be1 = np.asarray(inputs["be1"], np.float32)
    g2 = np.asarray(inputs["g2"], np.float32)
    be2 = np.asarray(inputs["be2"], np.float32)

    # (H, C, D) -> (C, H*D) with concat-head output order
    wq_flat = np.ascontiguousarray(Wq.transpose(1, 0, 2)).reshape(C, C)
    wk_flat = np.ascontiguousarray(Wk.transpose(1, 0, 2)).reshape(C, C)
    wv_flat = np.ascontiguousarray(Wv.transpose(1, 0, 2)).reshape(C, C)

    # fold LN1 gain into QKV weights, C**-0.5 into Q; LN biases into vectors
    wq_g = g1[:, None] * wq_flat * (C ** -0.5)
    wk_g = g1[:, None] * wk_flat
    wv_g = g1[:, None] * wv_flat
    w1_g = g2[:, None] * W1
    qb = (wq_flat.T @ be1) * (C ** -0.5)          # (C,)
    kb = wk_flat.T @ be1
    vb = wv_flat.T @ be1
    bo_eff = bo + Wo.T @ vb                        # V-bias folds through softmax
    b1_eff = b1 + W1.T @ be2

    sq = _pow2_scale(wq_g)
    sk = _pow2_scale(wk_g)
    sv = _pow2_scale(wv_g)
    so = _pow2_scale(Wo)
    s1 = _pow2_scale(w1_g)
    s2 = _pow2_scale(W2)

    sc = np.zeros((P, 8), np.float32)
    sc[:, SCQ], sc[:, SCK], sc[:, SCV] = 1.0 / sq, 1.0 / sk, 1.0 / sv
    sc[:, SCO], sc[:, SC1], sc[:, SC2] = 1.0 / so, 1.0 / s1, 1.0 / s2

    shared = {
        "wq8": _fp8(np.ascontiguousarray(
            (wq_g * sq).reshape(C2, 2, P, CC, P).transpose(2, 0, 1, 3, 4))),
        "wk8": _fp8(np.ascontiguousarray(
            (wk_g * sk).reshape(C2, 2, P, CC, P).transpose(2, 0, 1, 3, 4))),
        "wv8": _fp8(np.ascontiguousarray(
            (wv_g * sv).reshape(C2, 2, P, 2, TQ).transpose(2, 0, 1, 3, 4))),
        "wo8": _fp8(np.ascontiguousarray(
            (Wo * so).reshape(C2, 2, P, CC, P).transpose(2, 0, 1, 3, 4))),
        "w18": _fp8(np.ascontiguousarray(
            (w1_g * s1).reshape(C2, 2, P, 32, P).transpose(2, 3, 0, 1, 4))),
        "w28": _fp8(np.ascontiguousarray(
            (W2 * s2).reshape(16, 2, P, CC, P).transpose(2, 3, 0, 1, 4))),
        "sc_t": sc,
        "qb_t": np.ascontiguousarray(qb.reshape(CC, P).T),
        "kb_t": np.ascontiguousarray(kb.reshape(CC, P).T),
        "bo_t": np.ascontiguousarray(bo_eff.reshape(CC, P).T),
        "b1_t": np.ascontiguousarray(b1_eff.reshape(32, P).T),
        "b2_t": np.ascontiguousarray(b2.reshape(CC, P).T),
    }

    in_maps = []
    for c in range(8):
        b, a = c // 4, c % 4
        q0 = TQ * a
        perm = np.concatenate([np.arange(q0, q0 + TQ), np.arange(0, q0),
                               np.arange(q0 + TQ, T)])
        xbT = np.ascontiguousarray(x[b].T)                       # (C, T)
        xkvT = np.ascontiguousarray(
            xbT[:, perm].reshape(CC, P, T).transpose(1, 0, 2)).astype(bf16)
        xqT = np.ascontiguousarray(
            xbT[:, q0:q0 + TQ].reshape(CC, P, TQ).transpose(1, 0, 2))
        key_pos = perm  # original index of permuted key j
        mask01 = (key_pos[:, None] <= (q0 + np.arange(TQ))[None, :]).astype(
            np.float32)
        maskT = np.ascontiguousarray(
            mask01.reshape(NSCH, P, TQ).transpose(1, 0, 2)).astype(bf16)
        in_maps.append({
            "xkvT": xkvT,
            "xqT": xqT.astype(np.float32),
            "maskT": maskT,
            **shared,
        })
    return in_maps


def assemble_output(core_outs):
    """core_outs: list of 8 dicts with 'outT' (P, CC, TQ) fp32."""
    out = np.zeros((B, T, C), np.float32)
    for c in range(8):
        b, a = c // 4, c % 4
        y2 = core_outs[c]["outT"].transpose(1, 0, 2).reshape(C, TQ)
        out[b, TQ * a:TQ * (a + 1), :] = y2.T
    return out


def kernel(**inputs) -> np.ndarray:
    nc = build_nc()
    in_maps = make_in_maps(inputs)
    res = bass_utils.run_bass_kernel_spmd(nc, in_maps, core_ids=list(range(8)))
    return assemble_output(res.results)


if __name__ == "__main__":
    import reference
    inputs = {k: np.asarray(v) for k, v in reference.setup_inputs().items()}
    expected = np.asarray(reference.reference(**inputs))
    actual = kernel(**inputs)
    err = np.abs(actual - expected)
    print("absmax err:", err.max(), "scale:", np.abs(expected).max())
    print("rel fro:", np.linalg.norm(actual - expected) / np.linalg.norm(expected))


# revision 27
# speedup vs baseline: 2.0631x; 1.1260x over previous
"""Trainium2 Bass kernel for a dense transformer block (LN -> 16-head causal
attention -> residual -> LN -> FFN -> residual) on x:(2, 2048, 1024) fp32.

Sharding: 8 cores, zero collectives. Core c handles batch b=c//4 and 512
queries (chunk a=c%4). Each core recomputes full-sequence K/V for its batch.
Tokens are fed to each core PERMUTED so its own queries sit at positions
0-511; attention over keys is permutation-invariant given the matching 0/1
mask (per-core input data), so the compiled program is identical across
cores (SPMD) and the LN1 output can be sliced at a fixed offset for Q.

On-chip layout is feature-on-partition throughout. LayerNorm statistics come
from ones-vector matmuls; softmax uses a multiplicative 0/1 causal mask
applied AFTER exp (scores are tiny: |s|<~1, so exp never overflows and
masked lanes are zeroed exactly); the denominator comes from a ones column
appended to V. All projections (QKV/O/FFN) run in fp8e4m3 with DoubleRow
perf mode (2 contraction tiles per pass); weights are pre-scaled by a
power-of-two into fp8 range and descaled at PSUM readout. LN gains (g1/g2)
and the C**-0.5 score scale are folded into the weights host-side; LN biases
(be1/be2) are folded into per-feature bias vectors (the V-path bias commutes
through softmax since attention rows sum to 1, so it folds into bo).
Score/AV matmuls run in bf16. Exps process two heads per instruction from a
two-bank PSUM pair."""

import numpy as np
import ml_dtypes

import concourse.bass as bass
import concourse.tile as tile
from concourse import bacc, mybir
from concourse import bass_utils
from concourse.bass import ts

P = 128
B, T, C = 2, 2048, 1024
H, D = 16, 64
FF = 4 * C
CC = C // P          # 8 feature chunks
C2 = CC // 2         # 4 DoubleRow pair-chunks
TQ = 512             # queries per core
NSCH = T // P        # 16 key chunks
NTCH = T // TQ       # 4 token chunks of 512
EPS = 1e-5

bf16 = ml_dtypes.bfloat16
e4m3 = ml_dtypes.float8_e4m3

f32 = mybir.dt.float32
bf = mybir.dt.bfloat16
fp8 = mybir.dt.float8e4
AF = mybir.ActivationFunctionType
ALU = mybir.AluOpType
DR = mybir.MatmulPerfMode.DoubleRow

# sc_t column indices for the per-matrix fp8 descale factors
SCQ, SCK, SCV, SCO, SC1, SC2 = 0, 1, 2, 3, 4, 5


def _body(nc, tc, aps):
    (xkvT, xqT, maskT, wq8, wk8, wv8, wo8, w18, w28,
     sc_t, qb_t, kb_t, bo_t, b1_t, b1r_t, b2_t, outT) = aps

    import contextlib
    ctx = contextlib.ExitStack()
    with ctx:
        consts = ctx.enter_context(tc.tile_pool(name="consts", bufs=1))
        small = ctx.enter_context(tc.tile_pool(name="small", bufs=2))

        ones1 = consts.tile([P, 1], bf)
        nc.vector.memset(ones1, 1.0)
        onesr = consts.tile([1, TQ], bf)
        nc.vector.memset(onesr, 1.0)
        eps11 = consts.tile([1, 1], f32)
        nc.vector.memset(eps11, EPS)

        def load(pool, ap_dram, shape, dtype=f32, tag=None):
            t = pool.tile(list(shape), dtype, tag=tag or ap_dram.name)
            nc.sync.dma_start(t, ap_dram)
            return t

        sc_s = load(consts, sc_t, (P, 8))
        qb_s = load(consts, qb_t, (P, 8))
        kb_s = load(consts, kb_t, (P, 8))
        bo_s = load(consts, bo_t, (P, CC))
        b2_s = load(consts, b2_t, (P, CC))

        # ---- long-lived tensors (LIFO: outermost lives longest)
        ypool = ctx.enter_context(tc.tile_pool(name="ypool", bufs=1))
        y1s = [ypool.tile([P, TQ], f32, tag=f"y1_{mo}", name=f"y1_{mo}")
               for mo in range(CC)]

        opool = ctx.enter_context(tc.tile_pool(name="opool", bufs=1))
        OT = opool.tile([P, CC, TQ], fp8)
        xq_sb = opool.tile([P, CC, TQ], bf)
        wo_s = opool.tile([P, C2, 2, CC, P], fp8)

        kvq_ctx = contextlib.ExitStack()
        kvq = kvq_ctx.enter_context(tc.tile_pool(name="kvq", bufs=1))
        KTs = [kvq.tile([P, T], bf, tag=f"KT{p}", name=f"KT{p}")
               for p in range(CC)]
        QTs = [kvq.tile([P, TQ], bf, tag=f"QT{p}", name=f"QT{p}")
               for p in range(CC)]
        Vrs = [kvq.tile([P, H, 65], bf, tag=f"Vr{s}", name=f"Vr{s}")
               for s in range(NSCH)]
        for s in range(NSCH):
            nc.vector.memset(Vrs[s][:, :, 64:65], 1.0)

        # ---- LN1 over the full (permuted) sequence -> per-chunk fp8 tiles
        with tc.tile_pool(name="hpool", bufs=1) as hpool:
            hN8s = [hpool.tile([P, C2, 2, TQ], fp8, tag=f"hN8_{t}",
                              name=f"hN8_{t}") for t in range(NTCH)]

            pwq_ctx = __import__("contextlib").ExitStack()
            pw = pwq_ctx.enter_context(tc.tile_pool(name="pw", bufs=1))
            spsum = pwq_ctx.enter_context(
                tc.tile_pool(name="spsum", bufs=2, space="PSUM"))
            kpp = pwq_ctx.enter_context(
                tc.tile_pool(name="kpp", bufs=2, space="PSUM"))
            ppool = pwq_ctx.enter_context(
                tc.tile_pool(name="ppool", bufs=2, space="PSUM"))
            # ---- K, V, Q projections (fp8 DoubleRow)
            if True:

                def qproj(pair):
                    psum = ppool.tile([P, TQ], f32, tag="mm")
                    for c2 in range(C2):
                        nc.tensor.matmul(
                            psum, lhsT=wq_s[:, c2, :, pair, :],
                            rhs=hN8s[0][:, c2, :, :],
                            start=(c2 == 0), stop=(c2 == C2 - 1),
                            perf_mode=DR)
                    nc.vector.tensor_scalar(
                        QTs[pair], psum,
                        scalar1=sc_s[:, SCQ:SCQ + 1],
                        scalar2=qb_s[:, pair:pair + 1],
                        op0=ALU.mult, op1=ALU.add)

                def vproj(st, half):
                    psum = ppool.tile([P, TQ], f32, tag="mm")
                    for c2 in range(C2):
                        nc.tensor.matmul(
                            psum,
                            lhsT=hN8s[st // 4][:, c2, :, ts(st % 4, P)],
                            rhs=wv_s[:, c2, :, half, :],
                            start=(c2 == 0), stop=(c2 == C2 - 1),
                            perf_mode=DR)
                    nc.scalar.activation(
                        Vrs[st][:, half * 8:(half + 1) * 8, 0:64],
                        psum.rearrange("p (h d) -> p h d", d=D),
                        AF.Copy, scale=sc_s[:, SCV:SCV + 1])

                def kproj(pair, tch):
                    psum = kpp.tile([P, TQ], f32, tag="kmm")
                    for c2 in range(C2):
                        nc.tensor.matmul(
                            psum, lhsT=wk_s[:, c2, :, pair, :],
                            rhs=hN8s[tch][:, c2, :, :],
                            start=(c2 == 0), stop=(c2 == C2 - 1),
                            perf_mode=DR)
                    nc.vector.tensor_scalar(
                        KTs[pair][:, ts(tch, TQ)], psum,
                        scalar1=sc_s[:, SCK:SCK + 1],
                        scalar2=kb_s[:, pair:pair + 1],
                        op0=ALU.mult, op1=ALU.add)

                def proj(tch):
                    if tch == 0:
                        for pair in range(CC):
                            qproj(pair)
                    for st in range(4 * tch, 4 * tch + 4):
                        for half in range(2):
                            vproj(st, half)
                    for pair in range(CC):
                        kproj(pair, tch)
            with tc.tile_pool(name="xpool", bufs=4) as xpool, \
                 tc.tile_pool(name="lnt", bufs=2) as lnt, \
                 tc.tile_pool(name="lnbc", bufs=4) as lnbc, \
                 tc.tile_pool(name="lnb", bufs=1) as lnb:
                # x chunk DMAs first, then the QKV weights, so LN1 starts at
                # once and the weights arrive before the projections need them
                xcs, mb2s, ab2s = [], [], []
                for tch in range(NTCH):
                    xs = xpool.tile([P, CC, TQ], bf, tag="xc")
                    nc.sync.dma_start(xs, xkvT[:, :, ts(tch, TQ)])
                    xcs.append(xs)
                wk_s = load(pw, wk8, (P, C2, 2, CC, P), fp8)
                wv_s = load(pw, wv8, (P, C2, 2, 2, TQ), fp8)
                wq_s = load(pw, wq8, (P, C2, 2, CC, P), fp8)
                nc.sync.dma_start(xq_sb, xqT)
                nc.sync.dma_start(wo_s, wo8)
                def stats(tch):
                    xs = xcs[tch]
                    ps = spsum.tile([1, TQ], f32, tag="ps")
                    for cc in range(CC):
                        nc.tensor.matmul(ps, lhsT=ones1, rhs=xs[:, cc, :],
                                         start=(cc == 0), stop=(cc == CC - 1))
                    pq = spsum.tile([1, TQ], f32, tag="pq")
                    for c2 in range(C2):
                        sq2 = lnt.tile([P, 2, TQ], bf, tag="sq2")
                        x2 = xs[:, 2 * c2:2 * c2 + 2, :]
                        nc.scalar.activation(sq2, x2, AF.Square)
                        for i in range(2):
                            nc.tensor.matmul(pq, lhsT=ones1, rhs=sq2[:, i, :],
                                             start=(c2 == 0 and i == 0),
                                             stop=(c2 == C2 - 1 and i == 1))
                    m = lnb.tile([1, TQ], bf, tag="ln_m")
                    nc.vector.tensor_scalar_mul(m, ps, 1.0 / C)
                    q = lnb.tile([1, TQ], f32, tag="ln_q")
                    nc.vector.tensor_scalar_mul(q, pq, 1.0 / C)
                    msq = lnb.tile([1, TQ], f32, tag="ln_msq")
                    nc.vector.tensor_mul(msq, m, m)
                    nc.vector.tensor_tensor(q, q, msq, ALU.subtract)
                    sd = lnb.tile([1, TQ], f32, tag="ln_sd")
                    nc.scalar.activation(sd, q, AF.Sqrt, bias=eps11)
                    a_t = lnb.tile([1, TQ], bf, tag="ln_at")
                    with nc.allow_low_precision(reason="1/sigma in bf16"):
                        nc.vector.reciprocal(a_t, sd)
                    m_bc = lnbc.tile([P, TQ], bf, tag="ln_mbc")
                    nc.gpsimd.partition_broadcast(m_bc, m)
                    a_bc = lnbc.tile([P, TQ], bf, tag="ln_abc")
                    nc.gpsimd.partition_broadcast(a_bc, a_t)
                    mb2s.append(m_bc.unsqueeze(1).broadcast_to((P, 2, TQ)))
                    ab2s.append(a_bc.unsqueeze(1).broadcast_to((P, 2, TQ)))

                def norm(tch):
                    for c2 in range(C2):
                        eng = nc.gpsimd if c2 == 3 else nc.vector
                        t2 = lnt.tile([P, 2, TQ], bf, tag="ln_t2")
                        eng.tensor_tensor(
                            t2, xcs[tch][:, 2 * c2:2 * c2 + 2, :], mb2s[tch],
                            ALU.subtract)
                        eng.tensor_mul(hN8s[tch][:, c2, :, :], t2, ab2s[tch])

                # software pipeline: stats runs one chunk ahead of
                # normalize+projections so the in-order PE queue never parks
                stats(0)
                stats(1)
                norm(0)
                proj(0)
                stats(2)
                norm(1)
                proj(1)
                stats(3)
                norm(2)
                proj(2)
                norm(3)
                proj(3)

            pwq_ctx.close()

        # ---- attention: per head-pair, paired-bank scores -> exp -> 0/1 mask
        with tc.tile_pool(name="attp", bufs=1) as attp, \
             tc.tile_pool(name="pat", bufs=6) as pat, \
             tc.tile_pool(name="spool", bufs=2, space="PSUM") as spool, \
             tc.tile_pool(name="avp", bufs=2, space="PSUM") as avp:
            mask_sb = attp.tile([P, NSCH, TQ], bf)
            nc.sync.dma_start(mask_sb, maskT)
            for pr in range(CC):
                h0, h1 = 2 * pr, 2 * pr + 1
                av0 = avp.tile([P, TQ], f32, tag="av0")
                av1 = avp.tile([P, TQ], f32, tag="av1")
                for sch in range(NSCH):
                    sps = spool.tile([P, 2 * TQ], f32, tag="sps")
                    nc.tensor.matmul(sps[:, 0:TQ],
                                     lhsT=KTs[pr][0:64, ts(sch, P)],
                                     rhs=QTs[pr][0:64, :],
                                     start=True, stop=True)
                    nc.tensor.matmul(sps[:, TQ:2 * TQ],
                                     lhsT=KTs[pr][64:128, ts(sch, P)],
                                     rhs=QTs[pr][64:128, :],
                                     start=True, stop=True)
                    e2 = pat.tile([P, 2, TQ], bf, tag="e2")
                    nc.scalar.activation(
                        e2.rearrange("p a b -> p (a b)"), sps, AF.Exp)
                    em = pat.tile([P, 2, TQ], bf, tag="em")
                    m_b2 = mask_sb[:, sch, :].unsqueeze(1).broadcast_to(
                        (P, 2, TQ))
                    nc.vector.tensor_mul(em, e2, m_b2)
                    nc.tensor.matmul(av0[0:65, :], lhsT=Vrs[sch][:, h0, :],
                                     rhs=em[:, 0, :], start=(sch == 0),
                                     stop=(sch == NSCH - 1))
                    nc.tensor.matmul(av1[0:65, :], lhsT=Vrs[sch][:, h1, :],
                                     rhs=em[:, 1, :], start=(sch == 0),
                                     stop=(sch == NSCH - 1))
                for i, av in ((0, av0), (1, av1)):
                    zr = small.tile([1, TQ], bf, tag="zr")
                    with nc.allow_low_precision(reason="1/den bf16"):
                        nc.vector.reciprocal(zr, av[64:65, :])
                    zb = pat.tile([64, TQ], bf, tag="zb")
                    nc.gpsimd.partition_broadcast(zb, zr)
                    nc.vector.tensor_mul(
                        OT[64 * i:64 * i + 64, pr, :], av[0:64, :], zb)
        kvq_ctx.close()

        # ---- output projection + residual, LN2 stats interleaved; FFN
        with tc.tile_pool(name="ffp", bufs=1) as ffp, \
             tc.tile_pool(name="pft", bufs=3) as pft, \
             tc.tile_pool(name="pws", bufs=3) as pws, \
             tc.tile_pool(name="lnb2", bufs=2) as lnb2:
            import contextlib as _cl
            ps_ctx = _cl.ExitStack()
            spsum2 = ps_ctx.enter_context(
                tc.tile_pool(name="spsum2", bufs=1, space="PSUM"))
            ppoolO = ps_ctx.enter_context(
                tc.tile_pool(name="ppoolO", bufs=4, space="PSUM"))
            y1b = ffp.tile([P, CC, TQ], bf)
            h2N8 = ffp.tile([P, C2, 2, TQ], fp8)
            b1r_s = load(ffp, b1r_t, (1, FF), bf)
            w2_s = load(ffp, w28, (P, CC, 16, 2, P), fp8)

            ps2 = spsum2.tile([1, TQ], f32, tag="ps")
            pq2 = spsum2.tile([1, TQ], f32, tag="pq")
            for mo in range(CC):
                psum = ppoolO.tile([P, TQ], f32, tag="mm")
                for c2 in range(C2):
                    nc.tensor.matmul(
                        psum, lhsT=wo_s[:, c2, :, mo, :],
                        rhs=OT[:, 2 * c2:2 * c2 + 2, :],
                        start=(c2 == 0), stop=(c2 == C2 - 1),
                        perf_mode=DR)
                tb = pft.tile([P, TQ], bf, tag="ot_b")
                nc.scalar.activation(tb, psum, AF.Identity,
                                     bias=bo_s[:, mo:mo + 1],
                                     scale=sc_s[:, SCO:SCO + 1])
                nc.vector.tensor_tensor(y1s[mo], tb, xq_sb[:, mo, :], ALU.add)
                nc.scalar.copy(y1b[:, mo, :], y1s[mo])
                nc.tensor.matmul(ps2, lhsT=ones1, rhs=y1b[:, mo, :],
                                 start=(mo == 0), stop=(mo == CC - 1))
                sqm = pft.tile([P, TQ], bf, tag="sqm")
                nc.scalar.activation(sqm, y1b[:, mo, :], AF.Square)
                nc.tensor.matmul(pq2, lhsT=ones1, rhs=sqm,
                                 start=(mo == 0), stop=(mo == CC - 1))

            m = lnb2.tile([1, TQ], bf, tag="ln_m")
            nc.vector.tensor_scalar_mul(m, ps2, 1.0 / C)
            q = lnb2.tile([1, TQ], f32, tag="ln_q")
            nc.vector.tensor_scalar_mul(q, pq2, 1.0 / C)
            msq = lnb2.tile([1, TQ], f32, tag="ln_msq")
            nc.vector.tensor_mul(msq, m, m)
            nc.vector.tensor_tensor(q, q, msq, ALU.subtract)
            sd = lnb2.tile([1, TQ], f32, tag="ln_sd")
            nc.scalar.activation(sd, q, AF.Sqrt, bias=eps11)
            a_t = lnb2.tile([1, TQ], bf, tag="ln_at")
            with nc.allow_low_precision(reason="1/sigma in bf16"):
                nc.vector.reciprocal(a_t, sd)
            m_bc = lnb2.tile([P, TQ], bf, tag="ln_mbc")
            nc.gpsimd.partition_broadcast(m_bc, m)
            a_bc = lnb2.tile([P, TQ], bf, tag="ln_abc")
            nc.gpsimd.partition_broadcast(a_bc, a_t)
            m_b2 = m_bc.unsqueeze(1).broadcast_to((P, 2, TQ))
            a_b2 = a_bc.unsqueeze(1).broadcast_to((P, 2, TQ))
            for c2 in range(C2):
                eng = nc.gpsimd if c2 == 3 else nc.vector
                t2 = pft.tile([P, 2, TQ], bf, tag="ln_t2")
                eng.tensor_tensor(t2, y1b[:, 2 * c2:2 * c2 + 2, :],
                                  m_b2, ALU.subtract)
                eng.tensor_mul(h2N8[:, c2, :, :], t2, a_b2)

            ps_ctx.close()
            fp_ctx = _cl.ExitStack()
            fp1 = fp_ctx.enter_context(
                tc.tile_pool(name="fp1", bufs=2, space="PSUM"))
            ppool3 = fp_ctx.enter_context(
                tc.tile_pool(name="ppool3", bufs=2, space="PSUM"))
            zTs = [ffp.tile([P, 2, TQ], fp8, tag=f"zT{m}", name=f"zT{m}")
                   for m in range(16)]
            for m2 in range(16):
                psum = fp1.tile([P, 2 * TQ], f32, tag="m2")
                for half in range(2):
                    mm = 2 * m2 + half
                    w1t = pws.tile([P, C2, 2, P], fp8, tag="w1")
                    nc.sync.dma_start(w1t, w18[:, mm, :, :, :])
                    psl = psum[:, half * TQ:(half + 1) * TQ]
                    nc.tensor.matmul(psl, lhsT=b1r_s[:, ts(mm, P)], rhs=onesr,
                                     start=True, stop=False)
                    for c2 in range(C2):
                        nc.tensor.matmul(psl, lhsT=w1t[:, c2, :, :],
                                         rhs=h2N8[:, c2, :, :],
                                         start=False, stop=(c2 == C2 - 1),
                                         perf_mode=DR)
                nc.scalar.activation(
                    zTs[m2].rearrange("p a b -> p (a b)"), psum,
                    AF.Relu, scale=sc_s[:, SC1:SC1 + 1])

            for mo in range(CC):
                psum = ppool3.tile([P, TQ], f32, tag="mm")
                for f2 in range(16):
                    nc.tensor.matmul(psum, lhsT=w2_s[:, mo, f2, :, :],
                                     rhs=zTs[f2],
                                     start=(f2 == 0), stop=(f2 == 15),
                                     perf_mode=DR)
                tb = pft.tile([P, TQ], bf, tag="ffn_b")
                nc.scalar.activation(tb, psum, AF.Identity,
                                     bias=b2_s[:, mo:mo + 1],
                                     scale=sc_s[:, SC2:SC2 + 1])
                ot = pft.tile([P, TQ], f32, tag="ot")
                nc.vector.tensor_tensor(ot, tb, y1s[mo], ALU.add)
                nc.sync.dma_start(outT[:, mo, :], ot)
            fp_ctx.close()


_NC_CACHE = {}


def build_nc(reps=1):
    global _NC_CACHE
    if reps in _NC_CACHE:
        return _NC_CACHE[reps]
    nc = bacc.Bacc("TRN2", target_bir_lowering=False, debug=False,
                   enable_asserts=False, num_devices=8)

    def dram(name, shape, dtype, kind="ExternalInput"):
        return nc.dram_tensor(name, shape, dtype, kind=kind).ap()

    aps = (
        dram("xkvT", (P, CC, T), bf),
        dram("xqT", (P, CC, TQ), bf),
        dram("maskT", (P, NSCH, TQ), bf),
        dram("wq8", (P, C2, 2, CC, P), fp8),
        dram("wk8", (P, C2, 2, CC, P), fp8),
        dram("wv8", (P, C2, 2, 2, TQ), fp8),
        dram("wo8", (P, C2, 2, CC, P), fp8),
        dram("w18", (P, 32, C2, 2, P), fp8),
        dram("w28", (P, CC, 16, 2, P), fp8),
        dram("sc_t", (P, 8), f32),
        dram("qb_t", (P, CC), f32),
        dram("kb_t", (P, CC), f32),
        dram("bo_t", (P, CC), f32),
        dram("b1_t", (P, 32), f32),
        dram("b1r_t", (1, FF), bf),
        dram("b2_t", (P, CC), f32),
        dram("outT", (P, CC, TQ), f32, kind="ExternalOutput"),
    )
    with tile.TileContext(nc) as tc:
        for _ in range(reps):
            _body(nc, tc, aps)
    nc.compile()
    _NC_CACHE[reps] = nc
    return nc


def _pow2_scale(w, target=96.0):
    mx = float(np.abs(w).max())
    if mx == 0.0:
        return 1.0
    return float(2.0 ** np.floor(np.log2(target / mx)))


def _fp8(w):
    return np.clip(w, -240.0, 240.0).astype(e4m3)


def make_in_maps(inputs):
    """Build the 8 per-core input dicts from the full problem inputs."""
    x = np.asarray(inputs["x"], np.float32)
    Wq = np.asarray(inputs["Wq"], np.float32)
    Wk = np.asarray(inputs["Wk"], np.float32)
    Wv = np.asarray(inputs["Wv"], np.float32)
    Wo = np.asarray(inputs["Wo"], np.float32)
    W1 = np.asarray(inputs["W1"], np.float32)
    W2 = np.asarray(inputs["W2"], np.float32)
    bo = np.asarray(inputs["bo"], np.float32)
    b1 = np.asarray(inputs["b1"], np.float32)
    b2 = np.asarray(inputs["b2"], np.float32)
    g1 = np.asarray(inputs["g1"], np.float32)
    be1 = np.asarray(inputs["be1"], np.float32)
    g2 = np.asarray(inputs["g2"], np.float32)
    be2 = np.asarray(inputs["be2"], np.float32)

    # (H, C, D) -> (C, H*D) with concat-head output order
    wq_flat = np.ascontiguousarray(Wq.transpose(1, 0, 2)).reshape(C, C)
    wk_flat = np.ascontiguousarray(Wk.transpose(1, 0, 2)).reshape(C, C)
    wv_flat = np.ascontiguousarray(Wv.transpose(1, 0, 2)).reshape(C, C)

    # fold LN1 gain into QKV weights, C**-0.5 into Q; LN biases into vectors
    wq_g = g1[:, None] * wq_flat * (C ** -0.5)
    wk_g = g1[:, None] * wk_flat
    wv_g = g1[:, None] * wv_flat
    w1_g = g2[:, None] * W1
    qb = (wq_flat.T @ be1) * (C ** -0.5)          # (C,)
    kb = wk_flat.T @ be1
    vb = wv_flat.T @ be1
    bo_eff = bo + Wo.T @ vb                        # V-bias folds through softmax
    b1_eff = b1 + W1.T @ be2

    sq = _pow2_scale(wq_g)
    sk = _pow2_scale(wk_g)
    sv = _pow2_scale(wv_g)
    so = _pow2_scale(Wo)
    s1 = _pow2_scale(w1_g)
    s2 = _pow2_scale(W2)

    sc = np.zeros((P, 8), np.float32)
    sc[:, SCQ], sc[:, SCK], sc[:, SCV] = 1.0 / sq, 1.0 / sk, 1.0 / sv
    sc[:, SCO], sc[:, SC1], sc[:, SC2] = 1.0 / so, 1.0 / s1, 1.0 / s2

    shared = {
        "wq8": _fp8(np.ascontiguousarray(
            (wq_g * sq).reshape(C2, 2, P, CC, P).transpose(2, 0, 1, 3, 4))),
        "wk8": _fp8(np.ascontiguousarray(
            (wk_g * sk).reshape(C2, 2, P, CC, P).transpose(2, 0, 1, 3, 4))),
        "wv8": _fp8(np.ascontiguousarray(
            (wv_g * sv).reshape(C2, 2, P, 2, TQ).transpose(2, 0, 1, 3, 4))),
        "wo8": _fp8(np.ascontiguousarray(
            (Wo * so).reshape(C2, 2, P, CC, P).transpose(2, 0, 1, 3, 4))),
        "w18": _fp8(np.ascontiguousarray(
            (w1_g * s1).reshape(C2, 2, P, 32, P).transpose(2, 3, 0, 1, 4))),
        "w28": _fp8(np.ascontiguousarray(
            (W2 * s2).reshape(16, 2, P, CC, P).transpose(2, 3, 0, 1, 4))),
        "sc_t": sc,
        "qb_t": np.ascontiguousarray(qb.reshape(CC, P).T),
        "kb_t": np.ascontiguousarray(kb.reshape(CC, P).T),
        "bo_t": np.ascontiguousarray(bo_eff.reshape(CC, P).T),
        "b1_t": np.ascontiguousarray(b1_eff.reshape(32, P).T),
        "b1r_t": (b1_eff * s1).reshape(1, FF).astype(bf16),
        "b2_t": np.ascontiguousarray(b2.reshape(CC, P).T),
    }

    in_maps = []
    for c in range(8):
        b, a = c // 4, c % 4
        q0 = TQ * a
        perm = np.concatenate([np.arange(q0, q0 + TQ), np.arange(0, q0),
                               np.arange(q0 + TQ, T)])
        xbT = np.ascontiguousarray(x[b].T)                       # (C, T)
        xkvT = np.ascontiguousarray(
            xbT[:, perm].reshape(CC, P, T).transpose(1, 0, 2)).astype(bf16)
        xqT = np.ascontiguousarray(
            xbT[:, q0:q0 + TQ].reshape(CC, P, TQ).transpose(1, 0, 2))
        key_pos = perm  # original index of permuted key j
        mask01 = (key_pos[:, None] <= (q0 + np.arange(TQ))[None, :]).astype(
            np.float32)
        maskT = np.ascontiguousarray(
            mask01.reshape(NSCH, P, TQ).transpose(1, 0, 2)).astype(bf16)
        in_maps.append({
            "xkvT": xkvT,
            "xqT": xqT.astype(bf16),
            "maskT": maskT,
            **shared,
        })
    return in_maps


def assemble_output(core_outs):
    """core_outs: list of 8 dicts with 'outT' (P, CC, TQ) fp32."""
    out = np.zeros((B, T, C), np.float32)
    for c in range(8):
        b, a = c // 4, c % 4
        y2 = core_outs[c]["outT"].transpose(1, 0, 2).reshape(C, TQ)
        out[b, TQ * a:TQ * (a + 1), :] = y2.T
    return out


def kernel(**inputs) -> np.ndarray:
    nc = build_nc()
    in_maps = make_in_maps(inputs)
    res = bass_utils.run_bass_kernel_spmd(nc, in_maps, core_ids=list(range(8)))
    return assemble_output(res.results)


if __name__ == "__main__":
    import reference
    inputs = {k: np.asarray(v) for k, v in reference.setup_inputs().items()}
    expected = np.asarray(reference.reference(**inputs))
    actual = kernel(**inputs)
    err = np.abs(actual - expected)
    print("absmax err:", err.max(), "scale:", np.abs(expected).max())
    print("rel fro:", np.linalg.norm(actual - expected) / np.linalg.norm(expected))
